# revision 60
# baseline (speedup 1.0000x reference)
"""GATv2 (2-layer + linear head) Trainium2 Bass kernel, 8-core SPMD.

Strategy: edges sorted by dst, dst-range-partitioned across 8 cores; per
core the dst nodes are load-balanced across 98 windows of 128 (host-side
permutation, outputs unpermuted on host).  Edges are processed in batches
of 6 windows; per 128-edge group a weighted one-hot matrix scatters
[exp(e)*xl[src] | exp(e)] into a PSUM accumulator via one tensor-engine
matmul.  Node-level linears run data-parallel on device into fp16 gather
tables (256B row stride, chunk-balanced row permutation so each of the 4
int16-index chunks holds exactly N/4 rows incl. self-loop mass); the node
pass overlaps the edge phase via per-chunk dependency joiners (loads
dispatch on the Activation HWDGE queue, writes on SP).  Per-edge xl/xr
rows are fetched with dma_gather on 4 SWDGE queues, <=1024 descriptors
per call (hard ring limit), 4 sub-gathers filling one 32-group compute
block; gathered rows are narrow (132B/68B payloads - the 256B-multiple
elem_size restriction only applies to transpose-mode gathers, so
InstDMAGatherAnt is emitted directly).  The edge pipeline runs in fp16 on
the vector engine (2-4x perf modes, folded att-dot reduction); leaky-relu
runs on the activation engine (Prelu alpha=0.2 for edge scores, Prelu
0.01 for the layer-1 output); segment softmax uses unshifted exp
(shift-invariant; |e| <= ~5 here).  SPMD: one instruction stream for all
cores, plan padded to cross-core maxima."""
import sys
sys.path.insert(0, '/opt/trn_rl_repo')
import numpy as np

P = 128
N = 100000
F = 128
H1 = 64
H2 = 32
NDEV = 8
DN = N // NDEV            # 12500 dst nodes per device
NW = 98                   # dst windows per device
BW = 6                    # windows per batch
NCHUNK = 4                # tabL gather chunks (int16 idx limit 32767)
NPAD = (N + P - 1) // P * P        # 100096 tabL rows
CHUNKR = NPAD // NCHUNK   # 25024 rows per chunk; tabL row c*CHUNKR+q = node 4q+c
GSUB = 8                  # max 128-edge groups per dma_gather call (1024-desc ring)
MAXG = 32                 # groups per compute block (z-pipe/score granularity)
SCRATCH = 16384           # SWDGE ring: 16B per descriptor
DNP = NW * P              # 12544 padded dst rows
ROW = 128                 # fp16 table row elements (256B, gather stride)
NB = 16                   # node tiles per batched DMA


def _node_perm():
    """tabL row slot -> node id (chunk-balancing interleave)."""
    q = np.arange(CHUNKR)
    pi = np.zeros(NPAD, np.int64)
    for c in range(NCHUNK):
        pi[c * CHUNKR + q] = q * NCHUNK + c
    return np.minimum(pi, N - 1), pi < N   # clamp pad slots, validity mask


def _batches():
    out = []
    w = 0
    while w < NW:
        k = min(BW, NW - w)
        out.append((w, k))
        w += k
    return out


def _pack_idx16(idx):
    """idx: int array, len multiple of 128 -> [128, len//16] int16 tile data.
    Logical position i lives at [i % 16, i // 16], replicated over the 8
    16-partition groups (each SWDGE queue's Q7 pair reads its own group)."""
    n = len(idx)
    a = np.asarray(idx, np.int16).reshape(n // 16, 16).T  # [16, n//16]
    return np.tile(a, (8, 1))


def _build_plan(src, dst):
    """src/dst: int64 (dst-sorted, len E_tot).  Per-device dst load balancing
    via round-robin of degree-sorted dsts into windows; uniform instruction
    structure across cores (per-key group counts are cross-core maxima)."""
    batches = _batches()
    NBT = len(batches)
    w2b = np.zeros(NW, np.int64)
    w2i = np.zeros(NW, np.int64)
    for bi, (w0, k) in enumerate(batches):
        w2b[w0:w0 + k] = bi
        w2i[w0:w0 + k] = np.arange(k)

    counts = np.zeros((NDEV, NBT, NCHUNK, BW), np.int64)
    dev_edges = []
    perms = []
    for d in range(NDEV):
        lo, hi = np.searchsorted(dst, [DN * d, DN * (d + 1)])
        s = src[lo:hi]
        t0 = dst[lo:hi] - DN * d
        # window balancing: round-robin degree-sorted dsts into NW windows
        deg = np.bincount(t0, minlength=DN)
        order_d = np.argsort(-deg, kind="stable")
        perm = np.zeros(DN, np.int64)
        idxs = np.arange(DN)
        perm[order_d] = (idxs % NW) * P + idxs // NW
        perms.append(perm)
        t = perm[t0]
        ck = s % NCHUNK
        wloc = t // P
        bi = w2b[wloc]
        wi = w2i[wloc]
        key = (bi * NCHUNK + ck) * BW + wi
        np.add.at(counts[d].reshape(-1), key, 1)
        order = np.lexsort((t, ck, bi))
        dev_edges.append((s[order], t[order], key[order]))

    gu = (counts.max(axis=0) + P - 1) // P       # [NBT, NCHUNK, BW]
    layout = []
    icol_off = 0
    gcol_off = 0
    for bi, (w0, k) in enumerate(batches):
        gp = 0
        instrs = []        # (chunk, group offset in batch, G)
        groups_w = []      # window-in-batch per group
        for c in range(NCHUNK):
            run = 0
            for w in range(BW):
                run += gu[bi, c, w]
                groups_w += [w] * int(gu[bi, c, w])
            # split the run into even-sized calls (avoids tiny remainders)
            nsp = (run + MAXG - 1) // MAXG
            a = 0
            for si in range(nsp):
                g = run // nsp + (1 if si < run % nsp else 0)
                if g:
                    instrs.append((c, gp + a, g))
                    a += g
            gp += run
        icols = sum(16 * g for (_, _, g) in instrs)
        layout.append(dict(bi=bi, w0=w0, nw=k, GP=gp, instrs=instrs,
                           groups_w=groups_w, icol_off=icol_off,
                           gcol_off=gcol_off))
        icol_off += icols
        gcol_off += gp
    ICT, GCT = icol_off, gcol_off

    idx_all = np.zeros((NDEV, 128, ICT), np.int16)
    dstl_all = np.full((NDEV, 128, GCT), -1.0, np.float32)
    gu_flat = gu.reshape(-1)
    base_of_key = np.zeros(gu_flat.size + 1, np.int64)
    base_of_key[1:] = np.cumsum(gu_flat * P)
    gtot = int(gu.sum())
    for d in range(NDEV):
        s, t, key = dev_edges[d]
        kchange = np.r_[True, key[1:] != key[:-1]]
        runstart = np.maximum.accumulate(
            np.where(kchange, np.arange(len(key)), 0))
        within = np.arange(len(key)) - runstart
        slot = base_of_key[key] + within
        E_pad = gtot * P
        xl_rel = np.zeros(E_pad, np.int64)
        xr_rel = np.zeros(E_pad, np.int64)
        dstl_v = np.full(E_pad, -1.0, np.float32)
        xl_rel[slot] = s // NCHUNK
        wloc = t // P
        xr_rel[slot] = t - np.array([b[0] for b in batches])[w2b[wloc]] * P
        dstl_v[slot] = (t - wloc * P).astype(np.float32)
        for L in layout:
            bi, gp = L["bi"], L["GP"]
            e0 = base_of_key[(bi * NCHUNK) * BW]
            dv = dstl_v[e0:e0 + gp * P].reshape(gp, P).T
            dstl_all[d, :, L["gcol_off"]:L["gcol_off"] + gp] = dv
            ic = L["icol_off"]
            for (c, goff, G) in L["instrs"]:
                a0 = e0 + goff * P
                a1 = a0 + G * P
                idx_all[d, :, ic:ic + 8 * G] = _pack_idx16(xl_rel[a0:a1])
                idx_all[d, :, ic + 8 * G:ic + 16 * G] = _pack_idx16(xr_rel[a0:a1])
                ic += 16 * G
    return layout, ICT, GCT, idx_all, dstl_all, perms


def _dma_gather_any(gp, out_ap, in_ap, idxs_ap, num_idxs, elem_size,
                    elem_step, queue_num):
    """dma_gather with arbitrary gathered-row byte size (not a multiple of
    256B).  bass.dma_gather asserts elem_size_bytes % 256 == 0, but per the
    Q7 ucode that restriction only applies to transpose mode; non-transpose
    descriptors are byte-granular (only the table row STRIDE must be a
    multiple of 256B).  Emits InstDMAGatherAnt directly."""
    import concourse.mybir as mybir
    import concourse.ap_utils as ap_utils
    assert idxs_ap.dtype == mybir.dt.int16
    assert in_ap.dtype == out_ap.dtype
    assert ap_utils.ap_is_contiguous(in_ap.ap[1:])
    assert ap_utils.ap_is_contiguous(out_ap.ap[1:])
    assert ap_utils.ap_is_contiguous(idxs_ap.ap[1:])
    assert in_ap.ap[-1][1] == out_ap.ap[-1][1] == elem_size
    assert in_ap.ap[0][0] == elem_step
    assert num_idxs % P == 0
    assert out_ap.ap[0][1] * out_ap.ap[1][1] == num_idxs
    stride_bytes = elem_step * mybir.dt.size(in_ap.dtype)
    stride_bytes_256 = stride_bytes // 256
    assert stride_bytes_256 * 256 == stride_bytes and stride_bytes_256 < 256
    _in_ap = gp.lower_ap_dma(in_ap, for_custom_bir_dma=True)
    _idxs_ap = gp.lower_ap(idxs_ap)
    _out_ap = gp.lower_ap(out_ap)
    return gp.add_instruction(
        mybir.InstDMAGatherAnt(
            name=gp.bass.get_next_instruction_name(),
            ins=[*_in_ap, _idxs_ap,
                 gp.lower_val_access(gp.to_reg(num_idxs))],
            outs=[_out_ap],
            transpose=False,
            num_idxs=num_idxs,
            elem_size=elem_size,
            stride_bytes_256=stride_bytes_256,
            gen_mode=0,
            single_packet=True,
            queue_num=queue_num,
            sbuf_tokens_per_rank=0,
            sbuf_free_dim_per_rank=0,
            sbuf_free_dim_pad_per_rank=0,
            sbuf_byte_offset=0,
        ))


def _emit_node_pass(nc, npool, npsum, mybir, AL, add_dep_helper, src_dram,
                    wc, bias_bc, dst_dram, nrows, Cin, ncols,
                    boundaries=()):
    """Batched x @ W + b -> fp16 table rows [0:ncols].  src_dram [Cin,
    >=nrows] fp16 (transposed), dst_dram [>=nrows, ROW] fp16.  Loads
    dispatch on the Activation HWDGE queue, writes on SP (splits the
    sequencer dispatch cost).  For each row-threshold in `boundaries` a
    joiner nop is emitted as soon as the covering write is issued; returns
    the list of joiner instructions."""
    f32 = mybir.dt.float32
    f16 = mybir.dt.float16
    writes = []
    joiners = []
    bnd = list(boundaries)
    nt = (nrows + P - 1) // P
    blk = 0
    while blk < nt:
        k = min(NB, nt - blk)
        r0 = blk * P
        rows = min(nrows - r0, k * P)
        full = (rows == k * P)
        xt = npool.tile([Cin, NB * P], f16, tag="xt", name="xt")
        nc.scalar.dma_start(out=xt[:, :rows], in_=src_dram[:, r0:r0 + rows])
        ot = npool.tile([P, NB, ncols], f16, tag="ot", name="ot")
        i = 0
        while i < k:
            # pack up to 4 psum sub-tiles per bank so one vector op adds bias
            k4 = min(4, k - i)
            if not full:
                k4 = 1
            nv = min(P, rows - i * P)
            ps = npsum.tile([P, k4, ncols], f32, space="PSUM", tag="ps",
                            name="ps")
            for j in range(k4):
                nc.tensor.matmul(out=ps[:nv, j, :],
                                 lhsT=xt[:, (i + j) * P:(i + j) * P + nv],
                                 rhs=wc[:], start=True, stop=True)
            nc.vector.tensor_tensor(out=ot[:nv, i:i + k4, :], in0=ps[:nv, :, :],
                                    in1=bias_bc[:nv, :, :k4 * ncols].rearrange(
                                        "p one (f c) -> p (one f) c", c=ncols),
                                    op=AL.add)
            if not full:
                wi = nc.sync.dma_start(
                    out=dst_dram[r0 + i * P:r0 + i * P + nv, 0:ncols],
                    in_=ot[:nv, i, :])
                writes.append(wi)
            i += k4
        if full:
            dv = dst_dram[r0:r0 + k * P, 0:ncols].rearrange(
                "(b p) c -> p b c", p=P)
            wi = nc.sync.dma_start(out=dv, in_=ot[:, :k, :])
            writes.append(wi)
        blk += k
        while bnd and blk * P >= bnd[0]:
            bnd.pop(0)
            j = nc.sync.nop()
            for wi in writes:
                add_dep_helper(j.ins, wi.ins, sync=True,
                               reason="table rows ready")
            joiners.append(j)
    return joiners


def _build_gat_layer(Cin, Cout, layout, ICT, GCT, final_linear):
    """One dispatch: node-phase linears into fp16 gather tables, then the
    edge phase (gathers + segment softmax + one-hot scatter matmuls)."""
    import concourse.bacc as bacc
    import concourse.mybir as mybir
    import concourse.tile as tile
    from concourse.tile_rust import add_dep_helper

    f32 = mybir.dt.float32
    f16 = mybir.dt.float16
    i16 = mybir.dt.int16
    AL = mybir.AluOpType
    AF = mybir.ActivationFunctionType
    ncolsL = Cout + 2          # [xl | 1 | 0]
    C2 = Cout // 2
    C4 = Cout // 4

    nc = bacc.Bacc("TRN2", target_bir_lowering=False, debug=False,
                   num_swdge_queues=4, dynamic_dma_scratch_size=SCRATCH)
    t_xT = nc.dram_tensor("xT", [Cin, NPAD], f16, kind="ExternalInput")
    t_xdT = nc.dram_tensor("xdT", [Cin, DNP], f16, kind="ExternalInput")
    t_wl = nc.dram_tensor("wl", [Cin, ncolsL], f16, kind="ExternalInput")
    t_wr = nc.dram_tensor("wr", [Cin, Cout], f16, kind="ExternalInput")
    t_bl = nc.dram_tensor("bl", [128, 4 * ncolsL], f32, kind="ExternalInput")
    t_br = nc.dram_tensor("br", [128, 4 * Cout], f32, kind="ExternalInput")
    t_attb = nc.dram_tensor("attb", [128, MAXG * Cout], f16, kind="ExternalInput")
    if final_linear:
        t_wlinb = nc.dram_tensor("wlinb", [128, Cout], f32, kind="ExternalInput")
        t_blin2 = nc.dram_tensor("blin2", [128, 1], f32, kind="ExternalInput")
        t_out = nc.dram_tensor("out", [DNP, 1], f32, kind="ExternalOutput")
        OC = 1
        odt = f32
    else:
        t_b1o = nc.dram_tensor("b1o", [128, Cout], f32, kind="ExternalInput")
        t_out = nc.dram_tensor("h", [DNP, Cout], f16, kind="ExternalOutput")
        OC = Cout
        odt = f16
    t_eidx = nc.dram_tensor("eidx", [128, ICT], i16, kind="ExternalInput")
    t_dstl = nc.dram_tensor("dstl", [128, GCT], f32, kind="ExternalInput")
    tabL = nc.dram_tensor("tabL", [NPAD, ROW], f16, kind="Internal")
    tabR = nc.dram_tensor("tabR", [DNP, ROW], f16, kind="Internal")

    with tile.TileContext(nc) as tc:
        with tc.tile_pool(name="const", bufs=1) as cpool:
            iota = cpool.tile([P, P], f16)
            nc.gpsimd.iota(iota[:], pattern=[[1, P]], base=0, channel_multiplier=0,
                           allow_small_or_imprecise_dtypes=True)
            attb = cpool.tile([P, MAXG * Cout], f16)
            nc.sync.dma_start(out=attb[:], in_=t_attb[:])
            wl = cpool.tile([Cin, ncolsL], f16)
            wr = cpool.tile([Cin, Cout], f16)
            bl = cpool.tile([P, 1, 4 * ncolsL], f32)
            br = cpool.tile([P, 1, 4 * Cout], f32)
            nc.sync.dma_start(out=wl[:], in_=t_wl[:])
            nc.sync.dma_start(out=wr[:], in_=t_wr[:])
            nc.sync.dma_start(out=bl[:, 0, :], in_=t_bl[:])
            nc.sync.dma_start(out=br[:, 0, :], in_=t_br[:])
            if final_linear:
                wlinb = cpool.tile([P, Cout], f32)
                nc.sync.dma_start(out=wlinb[:], in_=t_wlinb[:])
                blin2 = cpool.tile([P, 1], f32)
                nc.sync.dma_start(out=blin2[:], in_=t_blin2[:])
            else:
                b1o = cpool.tile([P, Cout], f32)
                nc.sync.dma_start(out=b1o[:], in_=t_b1o[:])

            # ---------------- node phase + overlapped edge phase ----------
            with tc.tile_pool(name="nsb", bufs=3) as npool, \
                 tc.tile_pool(name="nps", bufs=2, space="PSUM") as npsum, \
                 tc.tile_pool(name="esb", bufs=3) as ep, \
                 tc.tile_pool(name="exl", bufs=16) as xp, \
                 tc.tile_pool(name="exr", bufs=16) as xrp, \
                 tc.tile_pool(name="ez", bufs=3) as zp, \
                 tc.tile_pool(name="etmp", bufs=6) as tp, \
                 tc.tile_pool(name="eps", bufs=6, space="PSUM") as eps:
                # per-chunk joiners: edge gathers wait only for the table
                # rows they read, so the edge phase overlaps the node phase
                joinR = _emit_node_pass(nc, npool, npsum, mybir, AL,
                                        add_dep_helper, t_xdT, wr, br, tabR,
                                        DNP, Cin, Cout, boundaries=[DNP])[0]
                joinL = _emit_node_pass(nc, npool, npsum, mybir, AL,
                                        add_dep_helper, t_xT, wl, bl, tabL,
                                        NPAD, Cin, ncolsL,
                                        boundaries=[(c + 1) * CHUNKR
                                                    for c in range(NCHUNK)])
                qn = 0
                for L in layout:
                    w0, nw, GP = L["w0"], L["nw"], L["GP"]
                    icols = sum(16 * g for (_, _, g) in L["instrs"])
                    idxT = ep.tile([P, icols], i16, tag="idx", name="idx")
                    nc.sync.dma_start(
                        out=idxT[:],
                        in_=t_eidx[:, L["icol_off"]:L["icol_off"] + icols])
                    dstlT = ep.tile([P, GP], f32, tag="dstl", name="dstl")
                    nc.sync.dma_start(
                        out=dstlT[:],
                        in_=t_dstl[:, L["gcol_off"]:L["gcol_off"] + GP])
                    eT = ep.tile([P, GP], f32, tag="e", name="e")
                    wT = ep.tile([P, GP], f32, tag="w", name="w")
                    acc = [eps.tile([P, Cout + 1], f32, space="PSUM", tag="acc",
                                    name=f"acc{i}") for i in range(nw)]
                    gw = L["groups_w"]
                    first = [True] * nw
                    lastg = [max((g for g in range(GP) if gw[g] == w), default=-1)
                             for w in range(nw)]
                    outt = ep.tile([P, BW, OC], odt, tag="outt", name="outt")

                    ic = 0
                    for (c, goff, G) in L["instrs"]:
                        xl = xp.tile([P, MAXG, ncolsL], f16, tag="xl", name="xl")
                        xr = xrp.tile([P, MAXG, Cout], f16, tag="xr", name="xr")
                        for j in range(0, G, GSUB):
                            g = min(GSUB, G - j)
                            nj = g * P
                            gl = _dma_gather_any(
                                nc.gpsimd, xl[:, j:j + g, :],
                                tabL[c * CHUNKR:, 0:ncolsL],
                                idxT[:, ic + 8 * j:ic + 8 * (j + g)],
                                nj, ncolsL, ROW, qn)
                            add_dep_helper(gl.ins, joinL[c].ins, sync=True,
                                           reason="gather after tabL chunk")
                            gr = _dma_gather_any(
                                nc.gpsimd, xr[:, j:j + g, :],
                                tabR[w0 * P:, 0:Cout],
                                idxT[:, ic + 8 * G + 8 * j:
                                     ic + 8 * G + 8 * (j + g)],
                                nj, Cout, ROW, (qn + 1) % 4)
                            add_dep_helper(gr.ins, joinR.ins, sync=True,
                                           reason="gather after tabR")
                            qn = (qn + 2) % 4
                        ic += 16 * G
                        z = zp.tile([P, MAXG * Cout], f16, tag="z", name="z")
                        za = zp.tile([P, MAXG * Cout], f16, tag="za", name="za")
                        zb = zp.tile([P, MAXG * C2], f16, tag="zb", name="zb")
                        z3 = z[:, :G * Cout].rearrange("p (g c) -> p g c", g=G)
                        za3 = za[:, :G * Cout].rearrange("p (g c) -> p g c", g=G)
                        nc.vector.tensor_tensor(out=z3, in0=xl[:, :G, 0:Cout],
                                                in1=xr[:, :G, 0:Cout], op=AL.add)
                        # leaky-relu(0.2) on the activation engine
                        nc.scalar.activation(out=za[:, :G * Cout],
                                             in_=z[:, :G * Cout],
                                             func=AF.Prelu, alpha=0.2)
                        nc.vector.tensor_tensor(
                            out=z3, in0=za3,
                            in1=attb[:, :G * Cout].rearrange("p (g c) -> p g c", g=G),
                            op=AL.mult)
                        zb3 = zb[:, :G * C2].rearrange("p (g c) -> p g c", g=G)
                        nc.vector.tensor_tensor(out=zb3, in0=z3[:, :, 0:C2],
                                                in1=z3[:, :, C2:Cout], op=AL.add)
                        zc3 = za[:, :G * C4].rearrange("p (g c) -> p g c", g=G)
                        nc.vector.tensor_tensor(out=zc3, in0=zb3[:, :, 0:C4],
                                                in1=zb3[:, :, C4:C2], op=AL.add)
                        nc.vector.tensor_reduce(out=eT[:, goff:goff + G], in_=zc3,
                                                axis=mybir.AxisListType.X, op=AL.add)
                        nc.scalar.activation(out=wT[:, goff:goff + G],
                                             in_=eT[:, goff:goff + G],
                                             func=AF.Exp)
                        for gi in range(G):
                            g = goff + gi
                            w = gw[g]
                            B = tp.tile([P, P], f16, tag="B", name="B")
                            nc.vector.tensor_scalar(out=B[:], in0=iota[:],
                                                    scalar1=dstlT[:, g:g + 1],
                                                    scalar2=wT[:, g:g + 1],
                                                    op0=AL.is_equal, op1=AL.mult)
                            nc.tensor.matmul(out=acc[w][:], lhsT=B[:],
                                             rhs=xl[:, gi, 0:Cout + 1],
                                             start=first[w],
                                             stop=(g == lastg[w]))
                            first[w] = False

                    for w in range(nw):
                        r = tp.tile([P, 1], f32, tag="r", name="r")
                        nc.vector.reciprocal(r[:], acc[w][:, Cout:Cout + 1])
                        if final_linear:
                            v = tp.tile([P, Cout], f32, tag="v", name="v")
                            nc.vector.tensor_tensor(out=v[:], in0=acc[w][:, :Cout],
                                                    in1=wlinb[:], op=AL.mult)
                            sv = tp.tile([P, 1], f32, tag="sv", name="sv")
                            nc.vector.tensor_reduce(out=sv[:], in_=v[:],
                                                    axis=mybir.AxisListType.X,
                                                    op=AL.add)
                            sv2 = tp.tile([P, 1], f32, tag="sv2", name="sv2")
                            nc.vector.tensor_scalar(out=sv2[:], in0=sv[:],
                                                    scalar1=r[:], scalar2=None,
                                                    op0=AL.mult)
                            nc.vector.tensor_tensor(out=outt[:, w, :], in0=sv2[:],
                                                    in1=blin2[:], op=AL.add)
                        else:
                            t1 = tp.tile([P, Cout], f32, tag="t1", name="t1")
                            nc.vector.tensor_scalar(out=t1[:], in0=acc[w][:, :Cout],
                                                    scalar1=r[:], scalar2=None,
                                                    op0=AL.mult)
                            t2 = tp.tile([P, Cout], f32, tag="t2", name="t2")
                            nc.vector.tensor_tensor(out=t2[:], in0=t1[:],
                                                    in1=b1o[:], op=AL.add)
                            # F.leaky_relu default 0.01 on activation engine
                            # (Prelu: shares the act-func table set with the
                            # edge-phase Prelu/Exp, avoiding table reloads)
                            nc.scalar.activation(out=outt[:, w, :], in_=t2[:],
                                                 func=AF.Prelu, alpha=0.01)
                    ov = t_out[w0 * P:(w0 + nw) * P, :].rearrange(
                        "(b p) c -> p b c", p=P)
                    nc.sync.dma_start(out=ov, in_=outt[:, :nw, :])
    nc.compile()
    return nc


_CACHE = {}


def kernel(x, edge_index, W1l, b1l, W1r, b1r, att1, bias1,
           W2l, b2l, W2r, b2r, att2, bias2, Wlin, blin):
    from concourse import bass_utils

    x = np.asarray(x, np.float32)
    edge_index = np.asarray(edge_index)
    src = np.concatenate([edge_index[0], np.arange(N, dtype=edge_index.dtype)]).astype(np.int64)
    dst = np.concatenate([edge_index[1], np.arange(N, dtype=edge_index.dtype)]).astype(np.int64)
    order = np.argsort(dst, kind="stable")
    src, dst = src[order], dst[order]

    layout, ICT, GCT, idx_all, dstl_all, perms = _build_plan(src, dst)

    def bcast(v, n=128):
        return np.tile(np.asarray(v, np.float32)[None, :], (n, 1))

    key = ("k", ICT, GCT)
    if key not in _CACHE:
        _CACHE[key] = (
            _build_gat_layer(F, H1, layout, ICT, GCT, final_linear=False),
            _build_gat_layer(H1, H2, layout, ICT, GCT, final_linear=True),
        )
    ncA, ncB = _CACHE[key]

    def prep_wl(W, b, Cout):
        Cin = W.shape[0]
        wl = np.zeros((Cin, Cout + 2), np.float16)
        wl[:, :Cout] = np.asarray(W, np.float16)
        bl = np.zeros((128, 4 * (Cout + 2)), np.float32)
        blr = bl.reshape(128, 4, Cout + 2)
        blr[:, :, :Cout] = np.asarray(b, np.float32)
        blr[:, :, Cout] = 1.0
        return wl, bl

    def prep_xd(xf16, d, perm):
        # device dst slice, window-permuted, transposed: [Cin, DNP]
        Cin = xf16.shape[1]
        xd = np.zeros((Cin, DNP), np.float16)
        xd[:, perm] = xf16[DN * d:DN * (d + 1)].T
        return xd

    # ---- dispatch A (layer 1) ----
    pi, _valid = _node_perm()
    xf16 = x.astype(np.float16)
    xT = np.ascontiguousarray(xf16[pi].T)
    wl1, bl1 = prep_wl(W1l, b1l, H1)
    attb1 = np.tile(np.asarray(att1, np.float16)[None, :], (128, MAXG))
    br1 = np.tile(np.asarray(b1r, np.float32)[None, :], (128, 4))
    in_maps = []
    for d in range(NDEV):
        in_maps.append(dict(
            xT=xT, xdT=prep_xd(xf16, d, perms[d]), wl=wl1,
            wr=np.asarray(W1r, np.float16),
            bl=bl1, br=br1, attb=attb1, b1o=bcast(bias1),
            eidx=idx_all[d], dstl=dstl_all[d]))
    resA = bass_utils.run_bass_kernel_spmd(ncA, in_maps, core_ids=list(range(NDEV)))
    h1 = np.empty((N, H1), np.float16)
    for d in range(NDEV):
        h1[DN * d:DN * (d + 1)] = resA.results[d]["h"][perms[d]]

    # ---- dispatch B (layer 2 + head) ----
    h1T = np.ascontiguousarray(h1[pi].T)
    wl2, bl2 = prep_wl(W2l, b2l, H2)
    attb2 = np.tile(np.asarray(att2, np.float16)[None, :], (128, MAXG))
    br2 = np.tile(np.asarray(b2r, np.float32)[None, :], (128, 4))
    wlinb = np.tile(np.asarray(Wlin, np.float32).reshape(1, H2), (128, 1))
    blin2 = float(np.asarray(bias2, np.float32) @ np.asarray(Wlin, np.float32).reshape(H2)
                  + np.asarray(blin, np.float32)[0])
    blin2t = np.full((128, 1), blin2, np.float32)
    in_maps = []
    for d in range(NDEV):
        in_maps.append(dict(
            xT=h1T, xdT=prep_xd(h1, d, perms[d]), wl=wl2,
            wr=np.asarray(W2r, np.float16),
            bl=bl2, br=br2, attb=attb2, wlinb=wlinb, blin2=blin2t,
            eidx=idx_all[d], dstl=dstl_all[d]))
    resB = bass_utils.run_bass_kernel_spmd(ncB, in_maps, core_ids=list(range(NDEV)))
    out = np.empty(N, np.float32)
    for d in range(NDEV):
        out[DN * d:DN * (d + 1)] = resB.results[d]["out"][perms[d], 0]

    kernel._last_exec_ns = (resA.exec_time_ns, resB.exec_time_ns)
    return out
